# revision 5
# baseline (speedup 1.0000x reference)
"""Trainium2 Bass kernel for nn_MultiHeadAttention_66202625900642.

Reference semantics (B=2, S=2048, E=1024, H=16 heads, D=64):
    qh = q @ Wq.T + bq   (same k, v)
    head split is a PLAIN RESHAPE (B, S, E) -> (B, H, S, D):
      head h of batch b = rows [128h, 128h+128) of qh[b] reinterpreted
      row-major as a (2048, 64) matrix (scrambled seq index s' = 16r + c).
    causal softmax over s', out @ Wp.T + bp.

Because the head split partitions the *sequence* rows, sharding each batch
into 4 row-blocks of 512 (= 4 heads) is fully local: 8 cores = 2 batches x 4
quarters, zero collectives. Weights are replicated.

Precision/perf scheme:
  - q/k/v projections run as fp8 DoubleRow matmuls (0.5 cycles/row, 2
    k-tiles per matmul) with 3-term error compensation:
        x @ W ~= x1@W1 + x2@W1 + x1@W2
    where x1 = e4m3(x), x2 = e5m2(x - x1), W1 = e4m3(32*W^T),
    W2 = e5m2(32*W^T - W1). All three terms share one PSUM accumulation at
    a uniform 32x scale. End-to-end rel err ~3e-3 (gate 2e-2).
  - The 32x rides through attention: scores psum is 1024x, folded into the
    exp scale (2^-13); vh is 32x so z is 32x, folded into Wp/32 host-side.
  - Biases: added at PSUM evacuation via partition-broadcast bias tiles
    (built once with 4 tiny PE matmuls), not per-group bias matmuls.
  - Attention (scores, exp, P@V) and the final projection stay fp16.

Per-core pipeline (fp32 PSUM accumulation everywhere):
  1. projections -> qh/kh/vh fp16 at 32x (fp8 weights/activations streamed)
  2. DRAM round-trip: qh/kh into per-pair [2048, 128] files (2 heads wide),
     vh natural; DMA-transpose pair files back as [128, 2048] = two heads'
     Q_hT/K_hT stacked; vh re-read as [128, 65] V' tiles (ones column -> row
     sums ride along the P^T @ V' matmul).
  3. attention per head pair: S^T blocks per (qc, j), one exp per psum
     group on ACT (scale 2^-13), causal triangles via gpsimd affine_select,
     P^T @ V' accumulates out^T[d, s'] + rowsum into per-chunk PSUM,
     evacuated to SBUF.
  4. normalization: reciprocal of rowsum, PE-broadcast (f32r K=1 matmul),
     fused into the stride-16 rearrange to final-projection layout.
  5. final projection -> y fp32 (Wp/32 + bp broadcast at evacuation).
"""

import numpy as np
import ml_dtypes

import concourse.bass as bass
import concourse.mybir as mybir
import concourse.tile as tile
from concourse import bacc
from concourse.bass_utils import run_bass_kernel_spmd

F16 = mybir.dt.float16
F32 = mybir.dt.float32
F8E4 = mybir.dt.float8e4   # e4m3
F8E5 = mybir.dt.float8e5   # e5m2
EXP = mybir.ActivationFunctionType.Exp
DR = mybir.MatmulPerfMode.DoubleRow
ADD = mybir.AluOpType.add

B, S, E = 2, 2048, 1024
SB = 512                # seq rows per core (= 4 heads)
N_CORES = 8
WSCALE = 32.0           # fp8 weight pre-scale
EXP_SCALE = 1.0 / (WSCALE * WSCALE * 8.0)   # 2^-13: undo q/k 32x, apply 1/sqrt(64)

E4 = ml_dtypes.float8_e4m3
E5 = ml_dtypes.float8_e5m2


def build(reps: int = 1, phases: int = 3):
    nc = bacc.Bacc(None, target_bir_lowering=False)

    ins = {}
    for nm in ("q", "k", "v"):
        ins[f"{nm}T1"] = nc.dram_tensor(f"{nm}T1", [E, SB], F8E4,
                                        kind="ExternalInput")
        ins[f"{nm}T2"] = nc.dram_tensor(f"{nm}T2", [E, SB], F8E5,
                                        kind="ExternalInput")
        ins[f"w{nm}1"] = nc.dram_tensor(f"w{nm}1", [E, E], F8E4,
                                        kind="ExternalInput")
        ins[f"w{nm}2"] = nc.dram_tensor(f"w{nm}2", [E, E], F8E5,
                                        kind="ExternalInput")
    wpT = nc.dram_tensor("wpT", [E, E], F16, kind="ExternalInput")
    bias = nc.dram_tensor("bias", [1, 4 * E], F16, kind="ExternalInput")
    y = nc.dram_tensor("y", [SB, E], F32, kind="ExternalOutput")

    with tile.TileContext(nc) as tc:
        with (
            tc.tile_pool(name="consts", bufs=1) as consts,
            tc.tile_pool(name="wpool", bufs=1) as wpool,
            tc.tile_pool(name="proj", bufs=2) as proj,
            tc.tile_pool(name="attn", bufs=1) as attn,
            tc.tile_pool(name="ptile", bufs=3) as ptile,
            tc.tile_pool(name="ypool", bufs=2) as ypool,
            tc.tile_pool(name="ps", bufs=3, space="PSUM") as ps,
            tc.tile_pool(name="dram", bufs=1, space="DRAM") as dram,
        ):
            # ---- constants: one DMA for all biases (values pre-scaled 32x
            # for q/k/v on host; bp at 1x)
            ones128 = consts.tile([1, 128], F16)
            nc.vector.memset(ones128, 1.0)
            bias_all = consts.tile([1, 4 * E], F16, name="bias_all")
            nc.sync.dma_start(out=bias_all, in_=bias[:, :])
            # partition-broadcast bias tiles, built once on PE
            bias_bc = consts.tile([128, 4, E], F16, name="bias_bc")
            for i in range(4):
                pbb = ps.tile([128, E], F32, tag="S", bufs=2, name=f"pbb{i}")
                for hf in range(2):
                    nc.tensor.matmul(
                        pbb[:, 512 * hf:512 * hf + 512], ones128[0:1, :],
                        bias_all[0:1, i * E + 512 * hf:i * E + 512 * hf + 512],
                        start=True, stop=True)
                nc.vector.tensor_copy(bias_bc[:, i, :], pbb)

            # ---- weight/activation tiles; q/k loaded now, v/p deferred ----
            w_sb, x_sb, dram_in = {}, {}, {}
            for nm in ("q", "k", "v"):
                w1_t = wpool.tile([128, 8, E], F8E4, name=f"w1_{nm}")
                w2_t = wpool.tile([128, 8, E], F8E5, name=f"w2_{nm}")
                x1_t = wpool.tile([128, 8, SB], F8E4, name=f"x1_{nm}")
                x2_t = wpool.tile([128, 8, SB], F8E5, name=f"x2_{nm}")
                w_sb[nm] = (w1_t, w2_t)
                x_sb[nm] = (x1_t, x2_t)
                dram_in[nm] = (ins[f"w{nm}1"], ins[f"w{nm}2"],
                               ins[f"{nm}T1"], ins[f"{nm}T2"])
            w_p = wpool.tile([128, 8, E], F16, name="w_p")
            w_sb["p"] = w_p

            def load_x(nm, half, var):
                xt = dram_in[nm][2 + var]
                xre = xt.ap().rearrange("(t p) s -> p t s", p=128)
                s0 = 256 * half
                nc.sync.dma_start(out=x_sb[nm][var][:, :, s0:s0 + 256],
                                  in_=xre[:, :, s0:s0 + 256])

            def load_w(nm, t, var):
                wt = dram_in[nm][var]
                wre = wt.ap().rearrange("(t p) f -> p t f", p=128)
                nc.sync.dma_start(out=w_sb[nm][var][:, t:t + 1, :],
                                  in_=wre[:, t:t + 1, :])

            def load_inputs(nm, eng=None):
                if nm == "p":
                    wre = wpT.ap().rearrange("(t p) f -> p t f", p=128)
                    for t2 in range(2):
                        nc.sync.dma_start(out=w_sb["p"][:, 4 * t2:4 * t2 + 4],
                                          in_=wre[:, 4 * t2:4 * t2 + 4])
                    return
                # v stream then the remaining activation halves
                load_x("v", 0, 0)
                load_x("v", 0, 1)
                for t in range(8):
                    load_w("v", t, 0)
                for t in range(8):
                    load_w("v", t, 1)
                for var in range(2):
                    load_x("q", 1, var)
                    load_x("k", 1, var)
                    load_x("v", 1, var)

            # first-use order: st0/st1 activations, then q/k weights per-t
            for var in range(2):
                load_x("q", 0, var)
                load_x("k", 0, var)
            for t in range(8):
                load_w("q", t, 0)
                load_w("k", t, 0)
            for t in range(8):
                load_w("q", t, 1)
                load_w("k", t, 1)

            # ---- DRAM scratch --------------------------------------------
            qkp = [dram.tile([2 * S, 128], F16, name=f"qkp{i}")
                   for i in range(2)]
            vh_d = dram.tile([SB, E], F16)
            xTd = dram.tile([4096, 128], F16)

            for rep in range(reps):
                _body(nc, tc, ps, proj, attn, ptile, ypool,
                      ones128, bias_bc, w_sb, x_sb, qkp, vh_d, xTd, y,
                      rep, phases, load_inputs if rep == 0 else None)
    nc.finalize()
    return nc


BIDX = {"q": 0, "v": 1, "p": 2, "k": 3}


def _body(nc, tc, ps, proj, attn, ptile, ypool, ones128,
          bias_bc, w_sb, x_sb, qkp, vh_d, xTd, y, rep, phases=3,
          load_inputs=None):
    xT2 = attn.tile([128, 8, SB], F16, tag="xT2", name=f"xT2_{rep}")
    if phases < 2:
        nc.vector.memset(xT2[:, 0, 0:1], 0.0)
    _xh_cache = {}

    def _xh(st, nm):
        xh = _xh_cache.get((st, nm))
        if xh is None:
            xh = proj.tile([128, E], F16, tag="xh", name=f"xh_{nm}{st}_{rep}")
            _xh_cache[(st, nm)] = xh
        return xh

    def _store_xh(st, nm, xh):
        if nm == "v":
            nc.sync.dma_start(out=vh_d[bass.ts(st, 128), :], in_=xh)
        else:
            tgt = qkp[st // 2]
            base = (0 if nm == "q" else S * 128) + 64 * (st % 2)
            out_ap = bass.AP(
                tgt.tensor, tgt.offset + base,
                [[2048, 128], [128, 16], [1, 64]])
            nc.sync.dma_start(
                out=out_ap, in_=xh.rearrange("r (c d) -> r c d", d=64))

    def _proj_mms(pp, st, nm, ch, term, tp, start, stop):
        # term 0: x1@W1, 1: x2@W1, 2: x1@W2 (DoubleRow over t-pair tp)
        xvar = 1 if term == 1 else 0
        wvar = 1 if term == 2 else 0
        nc.tensor.matmul(
            pp,
            x_sb[nm][xvar][:, 2 * tp:2 * tp + 2, bass.ts(st, 128)],
            w_sb[nm][wvar][:, 2 * tp:2 * tp + 2, bass.ts(ch, 512)],
            start=start, stop=stop, perf_mode=DR)

    def _evac(xh, ch, pp, nm):
        nc.vector.tensor_tensor(
            xh[:, bass.ts(ch, 512)], pp,
            bias_bc[:, BIDX[nm], bass.ts(ch, 512)], op=ADD)

    def project_unit(st, nm, ch):
        # one psum-group of the projection for (seq-tile st, proj nm, chunk ch)
        xh = _xh(st, nm)
        pp = ps.tile([128, 512], F32, tag="P1", bufs=2, name=f"pp{rep}")
        first = True
        for term in range(3):
            for tp in range(4):
                _proj_mms(pp, st, nm, ch, term, tp,
                          start=first, stop=(term == 2 and tp == 3))
                first = False
        _evac(xh, ch, pp, nm)
        if ch == 1:
            _store_xh(st, nm, xh)

    def project_chains(chains, tags):
        # interleaved t-outer accumulation chains chasing the per-t weight
        # DMAs; tags pick psum rings (idle attention slots borrowed early)
        pps, xhs = {}, {}
        for i, (st, nm) in enumerate(chains):
            xhs[(st, nm)] = _xh(st, nm)
            for ch in range(2):
                pp = ps.tile([128, 512], F32, tag=tags[i], bufs=2,
                             name=f"pp{nm}{st}{ch}_{rep}")
                pps[(st, nm, ch)] = pp
        # terms 0/1 use W1 (streamed first), term 2 uses W2
        for tp in range(4):
            for st, nm in chains:
                for ch in range(2):
                    for term in range(2):
                        _proj_mms(pps[(st, nm, ch)], st, nm, ch, term, tp,
                                  start=(tp == 0 and term == 0), stop=False)
        for tp in range(4):
            for st, nm in chains:
                for ch in range(2):
                    _proj_mms(pps[(st, nm, ch)], st, nm, ch, 2, tp,
                              start=False, stop=(tp == 3))
        for st, nm in chains:
            for ch in range(2):
                _evac(xhs[(st, nm)], ch, pps[(st, nm, ch)], nm)
            _store_xh(st, nm, xhs[(st, nm)])

    def attend_load(pair):
        QKT = ptile.tile([128, 2 * S], F16, tag="QKT", bufs=2,
                         name=f"QKT{pair}_{rep}")
        for hf in range(2):
            nc.sync.dma_start(out=QKT[:, 1024 * hf:1024 * hf + 1024],
                              in_=qkp[pair][1024 * hf:1024 * hf + 1024, :],
                              transpose=True)
        for hf in range(2):
            nc.sync.dma_start(out=QKT[:, S + 1024 * hf:S + 1024 * hf + 1024],
                              in_=qkp[pair][S + 1024 * hf:
                                            S + 1024 * hf + 1024, :],
                              transpose=True)
        return QKT[:, 0:S], QKT[:, S:2 * S]

    def attend(pair, loaded, fillers=(), tail_fill=((), ())):
        QT, KT = loaded
        fillers = list(fillers)
        vps = []
        for half in range(2):
            h = 2 * pair + half
            vp = ptile.tile([128, 16, 65], F16, tag="vp", bufs=4,
                            name=f"vp{h}_{rep}")
            v_src = bass.AP(vh_d.tensor, vh_d.offset + 128 * h * E,
                            [[64, 128], [8192, 16], [1, 64]])
            nc.sync.dma_start(out=vp[:, :, 0:64], in_=v_src)
            nc.vector.memset(vp[:, :, 64:65], 1.0)
            vps.append(vp)

        # per-head SBUF accumulators for out^T (+rowsum row 64)
        osb = [ptile.tile([65, 2048], F32, tag="osb", bufs=3,
                          name=f"osb{2 * pair + half}_{rep}")
               for half in range(2)]

        LAG = 2   # defer V-matmuls 2 groups behind S^T/exp (pt bufs cover it)
        pending = []

        def emit_vmms(ent):
            qc_, js_, pts_, psO_ = ent
            jmax_ = 4 * qc_ + 3
            for half in range(2):
                pt = pts_[half]
                for jj, j in enumerate(js_):
                    o = j - 4 * qc_
                    lo = 0 if o < 0 else 128 * o
                    nc.tensor.matmul(
                        psO_[half][:, lo:],
                        vps[half][:, j, :],
                        pt[:, 512 * jj + lo:512 * jj + 512],
                        start=(j == 0), stop=(j == jmax_))

        for qc in (1, 0, 2, 3):
            jmax = 4 * qc + 3
            psO = [ps.tile([65, 512], F32, tag="O", bufs=2,
                           name=f"psO{2 * pair + half}_{qc}_{rep}")
                   for half in range(2)]
            for j0 in range(0, jmax + 1, 2):
                js = [j for j in (j0, j0 + 1) if j <= jmax]
                lo0 = max(0, 128 * (js[0] - 4 * qc))
                pts = []
                for half in range(2):
                    psS = ps.tile([128, 1024], F32, tag="S", bufs=2,
                                  name=f"psS{half}_{qc}_{j0}_{rep}")
                    pt = ptile.tile([128, 1024], F16, tag="P", bufs=4,
                                    name=f"pt{half}_{qc}_{j0}_{rep}")
                    r0, r1 = 64 * half, 64 * half + 64
                    for jj, j in enumerate(js):
                        o = j - 4 * qc
                        lo = 0 if o < 0 else 128 * o
                        nc.tensor.matmul(
                            psS[:, 512 * jj + lo:512 * jj + 512],
                            KT[r0:r1, bass.ts(j, 128)],
                            QT[r0:r1, 512 * qc + lo:512 * qc + 512],
                            start=True, stop=True)
                    # one exp per group; stale lead-in cols are never read
                    nc.scalar.activation(pt[:, lo0:], psS[:, lo0:], EXP,
                                         scale=EXP_SCALE)
                    pts.append(pt)
                    for jj, j in enumerate(js):
                        o = j - 4 * qc
                        if o >= 0:
                            sl = pts[half][:, 512 * jj + 128 * o:
                                           512 * jj + 128 * o + 128]
                            nc.gpsimd.affine_select(
                                out=sl, in_=sl,
                                pattern=[[1, 128]],
                                compare_op=mybir.AluOpType.is_ge,
                                fill=0.0, base=0, channel_multiplier=-1)
                if fillers:
                    fillers.pop(0)()   # independent PE work while exp runs
                pending.append((qc, js, pts, psO))
                if len(pending) > LAG:
                    emit_vmms(pending.pop(0))
                if fillers:
                    fillers.pop(0)()
            # drain this qc's V-matmuls before evacuating its psO
            while pending:
                emit_vmms(pending.pop(0))
            for half in range(2):
                nc.vector.tensor_copy(osb[half][:, bass.ts(qc, 512)],
                                      psO[half])

            if qc in (0, 3):
                # normalize the finished s' segment [1024*seg, 1024*(seg+1))
                seg = 0 if qc == 0 else 1
                base = 1024 * seg
                for half in range(2):
                    h = 2 * pair + half
                    recip = ptile.tile([1, 1024], F32, tag="recip", bufs=4,
                                       name=f"recip{h}{seg}_{rep}")
                    nc.vector.reciprocal(recip,
                                         osb[half][64:65, base:base + 1024])
                    bsb = ptile.tile([64, 1024], F32, tag="bsb", bufs=2,
                                     name=f"bsb{h}{seg}_{rep}")
                    nc.gpsimd.partition_broadcast(bsb, recip)
                    o_re = osb[half][0:64, base:base + 1024].rearrange(
                        "p (r c) -> p c r", c=16)
                    b_re = bsb.rearrange("p (r c) -> p c r", c=16)
                    for t in range(8):
                        for h2 in range(2):
                            c = 2 * t + h2
                            nc.vector.tensor_tensor(
                                xT2[64 * h2:64 * h2 + 64, t,
                                    128 * h + 64 * seg:
                                    128 * h + 64 * seg + 64],
                                b_re[:, c, :], o_re[:, c, :],
                                op=mybir.AluOpType.mult)
                    if seg == 1:
                        for f in tail_fill[half]:
                            f()

    def final_unit(st, ch):
            py = ps.tile([128, 512], F32, tag="P1", bufs=2,
                         name=f"py{st}{ch}_{rep}")
            # xT2-dependent matmul first so the psum slot isn't grabbed early
            for t in range(8):
                nc.tensor.matmul(py,
                                 xT2[:, t, bass.ts(st, 128)],
                                 w_sb["p"][:, t, bass.ts(ch, 512)],
                                 start=(t == 0), stop=(t == 7))
            ysb = ypool.tile([128, 512], F32, tag="y",
                             name=f"ysb{st}{ch}_{rep}")
            nc.vector.tensor_tensor(ysb, py,
                                    bias_bc[:, 2, bass.ts(ch, 512)], op=ADD)
            nc.sync.dma_start(out=y[bass.ts(st, 128), bass.ts(ch, 512)],
                                in_=ysb)

    _f_state = {}

    def final_t(st, ch, t):
        ent = _f_state.get((st, ch))
        if ent is None:
            ent = ps.tile([128, 512], F32, tag="P1", bufs=2,
                          name=f"py{st}{ch}_{rep}")
            _f_state[(st, ch)] = ent
        py = ent
        nc.tensor.matmul(py,
                         xT2[:, t, bass.ts(st, 128)],
                         w_sb["p"][:, t, bass.ts(ch, 512)],
                         start=(t == 0), stop=(t == 7))
        if t == 7:
            ysb = ypool.tile([128, 512], F32, tag="y",
                             name=f"ysb{st}{ch}_{rep}")
            nc.vector.tensor_tensor(ysb, py,
                                    bias_bc[:, 2, bass.ts(ch, 512)], op=ADD)
            nc.sync.dma_start(out=y[bass.ts(st, 128), bass.ts(ch, 512)],
                              in_=ysb)

    def final(st):

        for ch in range(2):
            final_unit(st, ch)

    # pipeline: proj st0/st1 dense; pair-0 attention with proj st2/st3 as
    # PE fillers; pair-1 attention with final st0/st1 as fillers; tail.
    _xh_cache.clear()
    if phases < 2:
        if load_inputs is not None:
            load_inputs("v")
        for st in range(4):
            project_unit(st, "q", 0)
            project_unit(st, "q", 1)
            project_unit(st, "k", 0)
            project_unit(st, "k", 1)
            project_unit(st, "v", 0)
            project_unit(st, "v", 1)
        return
    # q/k projections of tiles 0/1 chase the per-t weight DMAs with
    # interleaved chains borrowing idle attention psum slots
    project_chains([(0, "q"), (0, "k"), (1, "q")], ["P1", "O", "S"])
    project_chains([(1, "k")], ["P1"])
    loaded0 = attend_load(0)
    if load_inputs is not None:
        load_inputs("v")
    project_chains([(0, "v"), (1, "v")], ["P1", "O"])
    loaded1_box = {}
    fill0 = [
        (lambda st=st, nm=nm, ch=ch: project_unit(st, nm, ch))
        for nm in ("q", "k") for st in (2, 3) for ch in range(2)
    ] + [
        lambda: loaded1_box.update(v=attend_load(1))
    ] + [
        (lambda st=st, ch=ch: project_unit(st, "v", ch))
        for st in (2, 3) for ch in range(2)
    ]
    attend(0, loaded0, fill0)
    if load_inputs is not None:
        load_inputs("p")
    loaded1 = loaded1_box["v"]
    if phases >= 3:
        fill1 = [
            (lambda st=st, ch=ch, t=t: final_t(st, ch, t))
            for st in (0, 1) for ch in range(2) for t in range(8)
        ]
        tails = ([(lambda ch=ch: final_unit(2, ch)) for ch in range(2)],
                 [(lambda ch=ch: final_unit(3, ch)) for ch in range(2)])
    else:
        fill1, tails = [], ((), ())
    attend(1, loaded1, fill1, tail_fill=tails)


# ---------------------------------------------------------------------------
# host side
# ---------------------------------------------------------------------------

_CACHE = {}


def _split8(x):
    x = np.asarray(x, np.float32)
    x1 = x.astype(E4)
    x2 = (x - x1.astype(np.float32)).astype(E5)
    return x1, x2


def _prep_inputs(q, k, v, Wq, bq, Wk, bk, Wv, bv, Wp, bp):
    shared = {}
    for nm, W in (("q", Wq), ("k", Wk), ("v", Wv)):
        Ws = np.ascontiguousarray(np.asarray(W, np.float32).T) * WSCALE
        w1, w2 = _split8(Ws)
        shared[f"w{nm}1"] = w1
        shared[f"w{nm}2"] = w2
    shared["wpT"] = np.ascontiguousarray(
        np.asarray(Wp, np.float32).T / WSCALE).astype(np.float16)
    shared["bias"] = np.concatenate([
        np.asarray(bq, np.float32) * WSCALE,
        np.asarray(bv, np.float32) * WSCALE,
        np.asarray(bp, np.float32),
        np.asarray(bk, np.float32) * WSCALE,
    ]).astype(np.float16)[None, :]
    in_maps = []
    for c in range(N_CORES):
        b, g = divmod(c, 4)
        rows = slice(SB * g, SB * (g + 1))
        m = dict(shared)
        for nm, x in (("q", q), ("k", k), ("v", v)):
            xT = np.ascontiguousarray(np.asarray(x[b, rows], np.float32).T)
            x1, x2 = _split8(xT)
            m[f"{nm}T1"] = x1
            m[f"{nm}T2"] = x2
        in_maps.append(m)
    return in_maps


def kernel(q, k, v, Wq, bq, Wk, bk, Wv, bv, Wp, bp):
    if "nc" not in _CACHE:
        _CACHE["nc"] = build()
    nc = _CACHE["nc"]
    in_maps = _prep_inputs(q, k, v, Wq, bq, Wk, bk, Wv, bv, Wp, bp)
    res = run_bass_kernel_spmd(nc, in_maps, core_ids=list(range(N_CORES)))
    out = np.empty((B, S, E), np.float32)
    for c in range(N_CORES):
        b, g = divmod(c, 4)
        out[b, SB * g:SB * (g + 1), :] = res.results[c]["y"]
    return out


if __name__ == "__main__":
    rng = np.random.default_rng(0)
    s = 1.0 / np.sqrt(E)
    ins = {
        "q": rng.standard_normal((B, S, E), dtype=np.float32),
        "k": rng.standard_normal((B, S, E), dtype=np.float32),
        "v": rng.standard_normal((B, S, E), dtype=np.float32),
        "Wq": rng.standard_normal((E, E), dtype=np.float32) * s,
        "bq": rng.standard_normal(E).astype(np.float32) * s,
        "Wk": rng.standard_normal((E, E), dtype=np.float32) * s,
        "bk": rng.standard_normal(E).astype(np.float32) * s,
        "Wv": rng.standard_normal((E, E), dtype=np.float32) * s,
        "bv": rng.standard_normal(E).astype(np.float32) * s,
        "Wp": rng.standard_normal((E, E), dtype=np.float32) * s,
        "bp": rng.standard_normal(E).astype(np.float32) * s,
    }
    out = kernel(**ins)
    print("kernel ran, out shape", out.shape, "mean", float(np.abs(out).mean()))
